# revision 2
# baseline (speedup 1.0000x reference)
"""Trainium2 Bass kernel for BSplineNN — v3.

Key change vs v2: the 4 knot-window SWDGE gathers are GONE. The window
kw[j] = t[i0+j] is extracted on-chip from the full knots tile via a one-hot
dot:  D[q] = indpad[q] - indpad[q+1]  is exactly [q == i0] (indpad has a
constant 1 at the left edge and 0 at the right edge, which makes the clamp
at i0=0 / i0=60 fall out automatically since ind[q] = [t_{q+4} <= x] is
non-increasing), then kw[j] = sum_q ktf[q+j] * D[q] — 8 masked dots split
DVE/Pool. Only the 4 coefficient gathers remain on the serial Pool desc-gen
chain (~1.04 us each), with element-granular indices into a wide [BC, N*C]
source view so each index emits ONE descriptor.

Sharding: pure data parallel, batch b = 4*p + g per core (512 batches).
"""

import numpy as np

import concourse.bacc as bacc
import concourse.bass as bass
import concourse.mybir as mybir
import concourse.tile as tile
from concourse.bass_utils import run_bass_kernel_spmd

B, N, C, T = 4096, 64, 256, 68   # batch, coef rows, channels, knots
K = 3                            # cubic
NCORES = 8
BC = B // NCORES                 # 512 batches per core
P = 128                          # partitions
G = BC // P                      # 4 batch-groups per partition
WROWS = K + 1                    # 4 gathered coef rows per batch
WK = 2 * K + 2                   # 8 window knots per batch
NMID = N - WROWS                 # 60 middle knots counted for i0
NQ = NMID + 1                    # 61 one-hot positions (i0 in [0, 60])
F32 = mybir.dt.float32
I32 = mybir.dt.int32
OP = mybir.AluOpType


def _emit_consts(nc, sb):
    """Loop-invariant index bases (f32 so the idx math stays float until the
    final int convert): bif = (4p+g)*N, and nothing else — kidx is gone."""
    bi = sb.tile([P, G], I32, tag="bi")
    nc.gpsimd.iota(out=bi[:], pattern=[[N, G]], base=0, channel_multiplier=N * G)
    bif256 = sb.tile([P, G], F32, tag="bif256")
    nc.vector.tensor_scalar(out=bif256[:], in0=bi[:], scalar1=float(C),
                            scalar2=None, op0=OP.mult)
    ones = sb.tile([P, G, WK - 1], F32, tag="ones")
    nc.vector.memset(ones[:], 1.0)
    return bif256, ones


def _emit(tc, nc, coef, knots, inpce, out, bif256, ones, tag=""):
    with tc.tile_pool(name=f"sb{tag}", bufs=1) as sb:
        # ---- load FULL knots (contiguous 1088 B per partition) + x ----
        ktf = sb.tile([P, G, T], F32)
        nc.sync.dma_start(out=ktf[:],
                          in_=knots.rearrange("(p g) t -> p g t", g=G))
        xt = sb.tile([P, G], F32)
        nc.scalar.dma_start(out=xt[:], in_=inpce.rearrange("(p g) o -> p (g o)", g=G))

        # ---- indicator with constant edge padding ----
        # indpad = [1, [t_4<=x], ..., [t_63<=x], 0]; i0 = sum of the middle.
        indpad = sb.tile([P, G, NMID + 2], F32)
        nc.gpsimd.memset(indpad[:][:, :, 0:1], 1.0)
        nc.gpsimd.memset(indpad[:][:, :, NMID + 1:NMID + 2], 0.0)
        nc.vector.tensor_tensor(out=indpad[:][:, :, 1:NMID + 1],
                                in0=xt[:].to_broadcast([P, G, NMID]),
                                in1=ktf[:][:, :, WROWS:N], op=OP.is_ge)
        i0f = sb.tile([P, G], F32)
        nc.vector.reduce_sum(out=i0f[:], in_=indpad[:][:, :, 1:NMID + 1],
                             axis=mybir.AxisListType.X)

        # ---- coef gather indices: gidx = ((4p+g)*N + i0) * C, via f32
        # (exact: max value 8.4M < 2^24) then one converting multiply ----
        gidx = sb.tile([P, G], I32)
        nc.vector.scalar_tensor_tensor(out=gidx[:], in0=i0f[:],
                                       scalar=float(C), in1=bif256[:],
                                       op0=OP.mult, op1=OP.add)

        # ---- the only SWDGE gathers: 4 coef chunks, wide-AP axis=1 ----
        gt = sb.tile([P, G, WROWS * C], F32)
        coef_wide = coef.rearrange("b n c -> b (n c)")
        order = []
        for g in range(G):
            order.append(nc.gpsimd.indirect_dma_start(
                out=gt[:][:, g, :], out_offset=None,
                in_=coef_wide,
                in_offset=bass.IndirectOffsetOnAxis(
                    ap=gidx[:][:, g:g + 1], axis=1)))
        for a, b in zip(order[1:], order):
            tile.add_dep_helper(a.ins, b.ins, sync=False,
                                reason="SWDGE emission order")

        # ---- knot windows on-chip: D = one-hot at i0, 8 masked mults and
        # ONE fused reduce, all DVE (keeps wts early; Pool stays on its
        # desc-gen chain). ----
        D = sb.tile([P, G, NQ], F32)
        nc.vector.tensor_tensor(out=D[:], in0=indpad[:][:, :, 0:NQ],
                                in1=indpad[:][:, :, 1:NQ + 1], op=OP.subtract)
        M = sb.tile([P, G, WK, NQ], F32)
        kw = sb.tile([P, G, WK], F32)
        for j in range(WK):
            nc.vector.tensor_tensor(out=M[:][:, :, j, :],
                                    in0=ktf[:][:, :, j:j + NQ],
                                    in1=D[:], op=OP.mult)
        nc.vector.reduce_sum(out=kw[:], in_=M[:], axis=mybir.AxisListType.X)

        # ---- windowed Cox-de Boor, all 4 groups in one pass, prep work
        # (indicators, knot diffs, v, c-mults) on Pool, recip/u/a/add on DVE ----
        xb = xt[:].to_broadcast([P, G, WK])
        xmt = sb.tile([P, G, WK], F32)
        nc.vector.tensor_tensor(out=xmt[:], in0=xb, in1=kw[:], op=OP.subtract)
        ind = sb.tile([P, G, WK], F32)
        nc.vector.tensor_tensor(out=ind[:], in0=xb, in1=kw[:], op=OP.is_ge)
        lvl = sb.tile([P, G, WK - 1], F32, tag="lvl0")
        nc.gpsimd.tensor_tensor(out=lvl[:], in0=ind[:][:, :, 0:WK - 1],
                                in1=ind[:][:, :, 1:WK], op=OP.subtract)
        us, vs = {}, {}
        for kk in (1, 2, 3):
            L1 = WK - kk          # number of u values = L + 1
            d = sb.tile([P, G, L1], F32, tag=f"d{kk}")
            nc.gpsimd.tensor_tensor(out=d[:], in0=kw[:][:, :, kk:kk + L1],
                                    in1=kw[:][:, :, 0:L1], op=OP.subtract)
            r = sb.tile([P, G, L1], F32, tag=f"r{kk}")
            nc.vector.reciprocal(out=r[:], in_=d[:])
            u = sb.tile([P, G, L1], F32, tag=f"u{kk}")
            nc.vector.tensor_tensor(out=u[:], in0=xmt[:][:, :, 0:L1],
                                    in1=r[:], op=OP.mult)
            v = sb.tile([P, G, L1], F32, tag=f"v{kk}")
            nc.gpsimd.tensor_tensor(out=v[:], in0=ones[:][:, :, 0:L1],
                                    in1=u[:], op=OP.subtract)
            us[kk], vs[kk] = u, v
        for kk in (1, 2, 3):
            L = WK - 1 - kk
            a = sb.tile([P, G, L], F32, tag=f"a{kk}")
            nc.vector.tensor_tensor(out=a[:], in0=us[kk][:][:, :, 0:L],
                                    in1=lvl[:][:, :, 0:L], op=OP.mult)
            c = sb.tile([P, G, L], F32, tag=f"c{kk}")
            nc.gpsimd.tensor_tensor(out=c[:], in0=vs[kk][:][:, :, 1:L + 1],
                                    in1=lvl[:][:, :, 1:L + 1], op=OP.mult)
            nxt = sb.tile([P, G, L], F32, tag=f"lvl{kk}")
            nc.vector.tensor_tensor(out=nxt[:], in0=a[:], in1=c[:], op=OP.add)
            lvl = nxt
        wts = lvl  # [P, G, 4] basis weights for rows i0..i0+3

        # ---- contraction: g0 = ACT Copy-scale mults + Pool adds (earliest
        # data; Pool/ACT have no per-partition-scalar ALU ops otherwise),
        # g1-g3 = DVE weighted chains. ----
        gtv = gt[:].rearrange("p g (d c) -> p g d c", d=WROWS)
        acc = sb.tile([P, G, C], F32)
        m0 = sb.tile([P, WROWS, C], F32)
        for d in range(WROWS):
            nc.scalar.activation(out=m0[:][:, d, :], in_=gtv[:, 0, d, :],
                                 func=mybir.ActivationFunctionType.Copy,
                                 scale=wts[:][:, 0, d:d + 1])
        s01 = sb.tile([P, C], F32)
        nc.gpsimd.tensor_tensor(out=s01[:], in0=m0[:][:, 0, :],
                                in1=m0[:][:, 1, :], op=OP.add)
        s23 = sb.tile([P, C], F32)
        nc.gpsimd.tensor_tensor(out=s23[:], in0=m0[:][:, 2, :],
                                in1=m0[:][:, 3, :], op=OP.add)
        nc.gpsimd.tensor_tensor(out=acc[:][:, 0, :], in0=s01[:], in1=s23[:],
                                op=OP.add)
        for g in range(1, G):
            nc.vector.tensor_scalar_mul(out=acc[:][:, g, :], in0=gtv[:, g, 0, :],
                                        scalar1=wts[:][:, g, 0:1])
            for d in range(1, WROWS):
                nc.vector.scalar_tensor_tensor(
                    out=acc[:][:, g, :], in0=gtv[:, g, d, :],
                    scalar=wts[:][:, g, d:d + 1], in1=acc[:][:, g, :],
                    op0=OP.mult, op1=OP.add)

        # ---- stores: groups 0-1 together, then g2 and g3 as each chain
        # finishes (g3 rides the last coef chunk; a lone 1 KB store is the
        # shortest possible tail) ----
        outv = out.rearrange("(p g) c -> p g c", g=G)
        nc.sync.dma_start(out=outv[:, 0:2, :], in_=acc[:][:, 0:2, :])
        nc.scalar.dma_start(out=outv[:, 2:3, :], in_=acc[:][:, 2:3, :])
        nc.sync.dma_start(out=outv[:, 3:4, :], in_=acc[:][:, 3:4, :])


def build_nc(reps=1):
    nc = bacc.Bacc("TRN2", target_bir_lowering=False, debug=False,
                   num_devices=NCORES)
    coef = nc.dram_tensor("coefficients", [BC, N, C], F32, kind="ExternalInput")
    knots = nc.dram_tensor("knots", [BC, T], F32, kind="ExternalInput")
    inpce = nc.dram_tensor("inpce", [BC, 1], F32, kind="ExternalInput")
    out = nc.dram_tensor("out", [BC, C], F32, kind="ExternalOutput")
    with tile.TileContext(nc) as tc:
        with tc.tile_pool(name="const", bufs=1) as cpool:
            bif256, ones = _emit_consts(nc, cpool)
            for r in range(reps):
                _emit(tc, nc, coef.ap(), knots.ap(), inpce.ap(), out.ap(),
                      bif256, ones, tag=str(r))
    nc.compile()
    return nc


def build_nc_loop(trip, unroll=4):
    """Kernel body in a hardware For_i loop — benchmarking only. Body is
    unrolled `unroll` times on distinct tiles; divide the slope by unroll."""
    nc = bacc.Bacc("TRN2", target_bir_lowering=False, debug=False,
                   num_devices=NCORES)
    coef = nc.dram_tensor("coefficients", [BC, N, C], F32, kind="ExternalInput")
    knots = nc.dram_tensor("knots", [BC, T], F32, kind="ExternalInput")
    inpce = nc.dram_tensor("inpce", [BC, 1], F32, kind="ExternalInput")
    out = nc.dram_tensor("out", [BC, C], F32, kind="ExternalOutput")
    with tile.TileContext(nc) as tc:
        with tc.tile_pool(name="const", bufs=1) as cpool:
            bif256, ones = _emit_consts(nc, cpool)
            with tc.For_i(0, trip, 1):
                for r in range(unroll):
                    _emit(tc, nc, coef.ap(), knots.ap(), inpce.ap(), out.ap(),
                          bif256, ones, tag=str(r))
    nc.compile()
    return nc


_NC_CACHE = None


def kernel(coefficients, knots, inpce, **run_kwargs):
    global _NC_CACHE
    if _NC_CACHE is None:
        _NC_CACHE = build_nc()
    nc = _NC_CACHE
    coefficients = np.ascontiguousarray(coefficients, dtype=np.float32)
    knots = np.ascontiguousarray(knots, dtype=np.float32)
    inpce = np.ascontiguousarray(inpce, dtype=np.float32)
    in_maps = []
    for k in range(NCORES):
        s = slice(k * BC, (k + 1) * BC)
        in_maps.append({"coefficients": coefficients[s],
                        "knots": knots[s],
                        "inpce": inpce[s]})
    res = run_bass_kernel_spmd(nc, in_maps, core_ids=list(range(NCORES)),
                               **run_kwargs)
    out = np.concatenate([res.results[k]["out"] for k in range(NCORES)], axis=0)
    if run_kwargs:
        return out, res
    return out


# revision 4
# speedup vs baseline: 1.5067x; 1.5067x over previous
"""Trainium2 Bass kernel for BSplineNN: cubic B-spline evaluation.

out[b, c] = sum_i coefficients[b, i, c] * N_{i,3}(x_b),  x_b = inpce[b, 0]

A cubic B-spline basis at one point has only 4 non-zero entries (rows
i0..i0+3, i0 = #{j in [4,64): t[j] <= x}), so per batch we fetch just the 4
relevant coefficient rows (4 KB) instead of all 64. Design notes, driven by
the TRN2 cost model (SWDGE desc-gen: 994 ns fixed + 0.34 ns/descriptor,
serial on the Pool engine; DMA ~22.5 B/ns; ~2 us HWDGE/sem latency per DMA):

* Only 4 SWDGE gathers (one per batch-group) with element-granular indices
  into a WIDE [BC, N*C] source view (axis=1 => index multiplier 1): the
  gathered 4 KB span always fits inside a source row, so each index emits
  ONE descriptor. (Indirect DMA consumes a single index per partition — a
  [P,G] index AP reads garbage on HW; and narrow [M,1] views fragment into
  one descriptor per element.)
* No knot-window gathers at all: kw[j] = t[i0+j] is selected ON-CHIP with a
  two-stage one-hot dot (coarse 16 blocks of 4, then fine 4), all exact
  integer-valued f32 compares. This removes 4 of the 8 desc-gens and their
  late DMA round trip.
* Group 0 is counted first so the serial desc-gen chain starts early; the
  basis runs under the coefficient DMA stream; contraction is spread over
  ACT (group-0 Copy-scale mults + Pool adds) and DVE (chains for groups
  1-3, last-arriving chunk last); three stores on alternating HWDGE rings.
* Pool's ALU has no comparison or per-partition-scalar ops (is_ge /
  TensorScalarPtr fail codegen) — it only gets plain add/sub/mult work.
* build_nc_loop software-pipelines the unrolled bodies (all loads, all
  fronts, all backs) so the in-order engine queues don't stall per body.

Sharding: pure data parallel, batch dim split across 8 cores (512 each).
Within a core, batch b = 4*p + g (p = partition 0..127, g = group 0..3).
"""

import numpy as np

import concourse.bacc as bacc
import concourse.bass as bass
import concourse.mybir as mybir
import concourse.tile as tile
from concourse.bass_utils import run_bass_kernel_spmd

B, N, C, T = 4096, 64, 256, 68   # batch, coef rows, channels, knots
K = 3                            # cubic
NCORES = 8
BC = B // NCORES                 # 512 batches per core
P = 128                          # partitions
G = BC // P                      # 4 batch-groups per partition
WROWS = K + 1                    # 4 gathered coef rows per batch
WK = 2 * K + 2                   # 8 window knots per batch
NMID = N - WROWS                 # 60 middle knots counted for i0
NQ = NMID + 1                    # 61 one-hot positions (i0 in [0, 60])
F32 = mybir.dt.float32
I32 = mybir.dt.int32
OP = mybir.AluOpType


def _emit_consts(nc, sb):
    """Loop-invariant index bases (f32 so the idx math stays float until the
    final int convert): bif = (4p+g)*N, and nothing else — kidx is gone."""
    bi = sb.tile([P, G], I32, tag="bi")
    nc.gpsimd.iota(out=bi[:], pattern=[[N, G]], base=0, channel_multiplier=N * G)
    bif256 = sb.tile([P, G], F32, tag="bif256")
    nc.vector.tensor_scalar(out=bif256[:], in0=bi[:], scalar1=float(C),
                            scalar2=None, op0=OP.mult)
    ones = sb.tile([P, G, WK - 1], F32, tag="ones")
    nc.vector.memset(ones[:], 1.0)
    agrid_i = sb.tile([P, G, 17], I32, tag="agrid_i")
    nc.gpsimd.iota(out=agrid_i[:], pattern=[[0, G], [4, 17]], base=0,
                   channel_multiplier=0)
    agrid = sb.tile([P, G, 17], F32, tag="agrid")
    nc.vector.tensor_copy(out=agrid[:], in_=agrid_i[:])
    bgrid_i = sb.tile([P, G, 5], I32, tag="bgrid_i")
    nc.gpsimd.iota(out=bgrid_i[:], pattern=[[0, G], [1, 5]], base=0,
                   channel_multiplier=0)
    bgrid = sb.tile([P, G, 5], F32, tag="bgrid")
    nc.vector.tensor_copy(out=bgrid[:], in_=bgrid_i[:])
    return bif256, ones, agrid, bgrid


def _emit_loads(tc, nc, coef, knots, inpce, out, bif256, ones, agrid, bgrid,
                tag=""):
    """Phase L: allocate this body's pool + tiles, issue the input DMAs."""
    pool_cm = tc.tile_pool(name=f"sb{tag}", bufs=1)
    sb = pool_cm.__enter__()
    st = {"pool_cm": pool_cm, "sb": sb,
          "args": (coef, knots, inpce, out, bif256, ones, agrid, bgrid)}
    # ---- load FULL knots (contiguous 1088 B per partition) + x ----
    ktf = sb.tile([P, G, T + 4], F32)
    nc.gpsimd.memset(ktf[:][:, :, T:T + 4], 0.0)
    kview = knots.rearrange("(p g) t -> p g t", g=G)
    nc.sync.dma_start(out=ktf[:][:, 0:1, 0:T], in_=kview[:, 0:1, :])
    nc.sync.dma_start(out=ktf[:][:, 1:G, 0:T], in_=kview[:, 1:G, :])
    xt = sb.tile([P, G], F32)
    nc.scalar.dma_start(out=xt[:], in_=inpce.rearrange("(p g) o -> p (g o)", g=G))
    st["ktf"], st["xt"] = ktf, xt
    return st


def _emit_front(tc, nc, st):
    """Phase F: count + gather indices + SWDGE gathers + window select +
    basis recurrence (everything up to the weights)."""
    sb = st["sb"]
    coef, knots, inpce, out, bif256, ones, agrid, bgrid = st["args"]
    ktf, xt = st["ktf"], st["xt"]
    if True:
        # ---- interval index i0 = #{j in [4,64): t[j] <= x} ----
        # group 0 counted FIRST so its gather index (and the serial Pool
        # desc-gen chain) starts ~0.6 us earlier; groups 1-3 follow while
        # desc-gen 0 runs.
        ind = sb.tile([P, G, NMID], F32)
        i0f = sb.tile([P, G], F32)
        gidx = sb.tile([P, G], I32)
        for gs in (slice(0, 1), slice(1, G)):
            w = gs.stop - gs.start
            nc.vector.tensor_tensor(out=ind[:][:, gs, :],
                                    in0=xt[:][:, gs].to_broadcast([P, w, NMID]),
                                    in1=ktf[:][:, gs, WROWS:N], op=OP.is_ge)
            nc.vector.reduce_sum(out=i0f[:][:, gs],
                                 in_=ind[:][:, gs, :],
                                 axis=mybir.AxisListType.X)
            # gidx = ((4p+g)*N + i0) * C via f32 (exact: max 8.4M < 2^24)
            nc.vector.scalar_tensor_tensor(out=gidx[:][:, gs], in0=i0f[:][:, gs],
                                           scalar=float(C), in1=bif256[:][:, gs],
                                           op0=OP.mult, op1=OP.add)

        # ---- the only SWDGE gathers: 4 coef chunks, wide-AP axis=1 ----
        gt = sb.tile([P, G, WROWS * C], F32)
        coef_wide = coef.rearrange("b n c -> b (n c)")
        order = []
        for g in range(G):
            order.append(nc.gpsimd.indirect_dma_start(
                out=gt[:][:, g, :], out_offset=None,
                in_=coef_wide,
                in_offset=bass.IndirectOffsetOnAxis(
                    ap=gidx[:][:, g:g + 1], axis=1)))
        for a, b in zip(order[1:], order):
            tile.add_dep_helper(a.ins, b.ins, sync=False,
                                reason="SWDGE emission order")

        # ---- knot windows on-chip, two-stage one-hot select (all DVE).
        # Stage 1: a0 = floor(i0/4); Da = one-hot over the 16 4-aligned
        # blocks; za[m] = sum_a ktf[4a+m]*Da[a] (m = 0..10) lifts an 11-knot
        # aligned window. Stage 2: b0 = i0 - 4*a0; Db = one-hot over 4;
        # kw[j] = sum_b za[b+j]*Db[b]. Indicator math is exact (integer-valued
        # f32 compares); the ktf pad is zeroed so Da's zero terms stay finite.
        inda = sb.tile([P, G, 17], F32)
        nc.vector.tensor_tensor(out=inda[:], in0=i0f[:].to_broadcast([P, G, 17]),
                                in1=agrid[:], op=OP.is_ge)
        Da = sb.tile([P, G, 16], F32)
        nc.vector.tensor_tensor(out=Da[:], in0=inda[:][:, :, 0:16],
                                in1=inda[:][:, :, 1:17], op=OP.subtract)
        a0f = sb.tile([P, G], F32)
        nc.vector.reduce_sum(out=a0f[:], in_=inda[:][:, :, 1:17],
                             axis=mybir.AxisListType.X)
        b0f = sb.tile([P, G], F32)
        nc.vector.scalar_tensor_tensor(out=b0f[:], in0=a0f[:], scalar=-4.0,
                                       in1=i0f[:], op0=OP.mult, op1=OP.add)
        indb = sb.tile([P, G, 5], F32)
        nc.vector.tensor_tensor(out=indb[:], in0=b0f[:].to_broadcast([P, G, 5]),
                                in1=bgrid[:], op=OP.is_ge)
        Db = sb.tile([P, G, 4], F32)
        nc.vector.tensor_tensor(out=Db[:], in0=indb[:][:, :, 0:4],
                                in1=indb[:][:, :, 1:5], op=OP.subtract)
        vv = ktf[:].rearrange("p g (a r) -> p g r a", r=4)  # [P, G, 4, 18]
        ZA = sb.tile([P, G, 11, 16], F32)
        for m in range(11):
            nc.vector.tensor_tensor(
                out=ZA[:][:, :, m, :],
                in0=vv[:, :, m % 4, m // 4:m // 4 + 16],
                in1=Da[:], op=OP.mult)
        za = sb.tile([P, G, 11], F32)
        nc.vector.reduce_sum(out=za[:], in_=ZA[:], axis=mybir.AxisListType.X)
        KW4 = sb.tile([P, G, WK, 4], F32)
        for b in range(4):
            nc.vector.tensor_tensor(
                out=KW4[:][:, :, :, b],
                in0=za[:][:, :, b:b + WK],
                in1=Db[:][:, :, b:b + 1].to_broadcast([P, G, WK]),
                op=OP.mult)
        kw = sb.tile([P, G, WK], F32)
        nc.vector.reduce_sum(out=kw[:], in_=KW4[:], axis=mybir.AxisListType.X)

        # ---- windowed Cox-de Boor, all 4 groups in one pass, prep work
        # (indicators, knot diffs, v, c-mults) on Pool, recip/u/a/add on DVE ----
        xb = xt[:].to_broadcast([P, G, WK])
        xmt = sb.tile([P, G, WK], F32)
        nc.vector.tensor_tensor(out=xmt[:], in0=xb, in1=kw[:], op=OP.subtract)
        ind = sb.tile([P, G, WK], F32)
        nc.vector.tensor_tensor(out=ind[:], in0=xb, in1=kw[:], op=OP.is_ge)
        lvl = sb.tile([P, G, WK - 1], F32, tag="lvl0")
        nc.vector.tensor_tensor(out=lvl[:], in0=ind[:][:, :, 0:WK - 1],
                                in1=ind[:][:, :, 1:WK], op=OP.subtract)
        us, vs = {}, {}
        for kk in (1, 2, 3):
            L1 = WK - kk          # number of u values = L + 1
            d = sb.tile([P, G, L1], F32, tag=f"d{kk}")
            nc.gpsimd.tensor_tensor(out=d[:], in0=kw[:][:, :, kk:kk + L1],
                                    in1=kw[:][:, :, 0:L1], op=OP.subtract)
            r = sb.tile([P, G, L1], F32, tag=f"r{kk}")
            nc.vector.reciprocal(out=r[:], in_=d[:])
            u = sb.tile([P, G, L1], F32, tag=f"u{kk}")
            nc.vector.tensor_tensor(out=u[:], in0=xmt[:][:, :, 0:L1],
                                    in1=r[:], op=OP.mult)
            v = sb.tile([P, G, L1], F32, tag=f"v{kk}")
            nc.gpsimd.tensor_tensor(out=v[:], in0=ones[:][:, :, 0:L1],
                                    in1=u[:], op=OP.subtract)
            us[kk], vs[kk] = u, v
        for kk in (1, 2, 3):
            L = WK - 1 - kk
            a = sb.tile([P, G, L], F32, tag=f"a{kk}")
            nc.vector.tensor_tensor(out=a[:], in0=us[kk][:][:, :, 0:L],
                                    in1=lvl[:][:, :, 0:L], op=OP.mult)
            c = sb.tile([P, G, L], F32, tag=f"c{kk}")
            nc.vector.tensor_tensor(out=c[:], in0=vs[kk][:][:, :, 1:L + 1],
                                    in1=lvl[:][:, :, 1:L + 1], op=OP.mult)
            nxt = sb.tile([P, G, L], F32, tag=f"lvl{kk}")
            nc.vector.tensor_tensor(out=nxt[:], in0=a[:], in1=c[:], op=OP.add)
            lvl = nxt
        wts = lvl  # [P, G, 4] basis weights for rows i0..i0+3
    st["gt"], st["wts"] = gt, wts
    return st


def _emit_back(tc, nc, st):
    """Phase B: contraction + stores; releases the body's pool."""
    sb = st["sb"]
    coef, knots, inpce, out, bif256, ones, agrid, bgrid = st["args"]
    gt, wts = st["gt"], st["wts"]
    if True:
        # ---- contraction: g0 = ACT Copy-scale mults + Pool adds (earliest
        # chunk, runs beside the DVE chains); g1-g3 = DVE weighted chains in
        # data-arrival order (g3 last). ----
        gtv = gt[:].rearrange("p g (d c) -> p g d c", d=WROWS)
        acc = sb.tile([P, G, C], F32)
        m0 = sb.tile([P, WROWS, C], F32)
        for d in range(WROWS):
            nc.scalar.activation(out=m0[:][:, d, :], in_=gtv[:, 0, d, :],
                                 func=mybir.ActivationFunctionType.Copy,
                                 scale=wts[:][:, 0, d:d + 1])
        s01 = sb.tile([P, C], F32)
        nc.gpsimd.tensor_tensor(out=s01[:], in0=m0[:][:, 0, :],
                                in1=m0[:][:, 1, :], op=OP.add)
        s23 = sb.tile([P, C], F32)
        nc.gpsimd.tensor_tensor(out=s23[:], in0=m0[:][:, 2, :],
                                in1=m0[:][:, 3, :], op=OP.add)
        nc.gpsimd.tensor_tensor(out=acc[:][:, 0, :], in0=s01[:], in1=s23[:],
                                op=OP.add)
        for g in (1, 2, 3):
            nc.vector.tensor_scalar_mul(out=acc[:][:, g, :], in0=gtv[:, g, 0, :],
                                        scalar1=wts[:][:, g, 0:1])
            for d in range(1, WROWS):
                nc.vector.scalar_tensor_tensor(
                    out=acc[:][:, g, :], in0=gtv[:, g, d, :],
                    scalar=wts[:][:, g, d:d + 1], in1=acc[:][:, g, :],
                    op0=OP.mult, op1=OP.add)

        # ---- stores: groups 0-1 together, then g2 and g3 as each chain
        # finishes (g3 rides the last coef chunk; a lone 1 KB store is the
        # shortest possible tail) ----
        outv = out.rearrange("(p g) c -> p g c", g=G)
        nc.sync.dma_start(out=outv[:, 1:3, :], in_=acc[:][:, 1:3, :])
        nc.scalar.dma_start(out=outv[:, 0:1, :], in_=acc[:][:, 0:1, :])
        nc.sync.dma_start(out=outv[:, 3:4, :], in_=acc[:][:, 3:4, :])


def _release(sts):
    """Pop the bodies' tile pools in reverse (stack) order."""
    for st in reversed(sts):
        st["pool_cm"].__exit__(None, None, None)


def _emit(tc, nc, coef, knots, inpce, out, bif256, ones, agrid, bgrid, tag=""):
    st = _emit_loads(tc, nc, coef, knots, inpce, out, bif256, ones, agrid,
                     bgrid, tag=tag)
    _emit_front(tc, nc, st)
    _emit_back(tc, nc, st)
    _release([st])


def build_nc(reps=1):
    nc = bacc.Bacc("TRN2", target_bir_lowering=False, debug=False,
                   num_devices=NCORES)
    coef = nc.dram_tensor("coefficients", [BC, N, C], F32, kind="ExternalInput")
    knots = nc.dram_tensor("knots", [BC, T], F32, kind="ExternalInput")
    inpce = nc.dram_tensor("inpce", [BC, 1], F32, kind="ExternalInput")
    out = nc.dram_tensor("out", [BC, C], F32, kind="ExternalOutput")
    with tile.TileContext(nc) as tc:
        with tc.tile_pool(name="const", bufs=1) as cpool:
            bif256, ones, agrid, bgrid = _emit_consts(nc, cpool)
            for r in range(reps):
                _emit(tc, nc, coef.ap(), knots.ap(), inpce.ap(), out.ap(),
                      bif256, ones, agrid, bgrid, tag=str(r))
    nc.compile()
    return nc


def build_nc_loop(trip, unroll=4):
    """Kernel body in a hardware For_i loop — benchmarking only. Body is
    unrolled `unroll` times on distinct tiles; divide the slope by unroll."""
    nc = bacc.Bacc("TRN2", target_bir_lowering=False, debug=False,
                   num_devices=NCORES)
    coef = nc.dram_tensor("coefficients", [BC, N, C], F32, kind="ExternalInput")
    knots = nc.dram_tensor("knots", [BC, T], F32, kind="ExternalInput")
    inpce = nc.dram_tensor("inpce", [BC, 1], F32, kind="ExternalInput")
    out = nc.dram_tensor("out", [BC, C], F32, kind="ExternalOutput")
    with tile.TileContext(nc) as tc:
        with tc.tile_pool(name="const", bufs=1) as cpool:
            bif256, ones, agrid, bgrid = _emit_consts(nc, cpool)
            with tc.For_i(0, trip, 1):
                # software-pipeline the unrolled bodies: all loads, then all
                # fronts, then all backs, so the in-order engine queues never
                # stall on one body's DMA waits while another body has work.
                sts = [_emit_loads(tc, nc, coef.ap(), knots.ap(), inpce.ap(),
                                   out.ap(), bif256, ones, agrid, bgrid,
                                   tag=str(r))
                       for r in range(unroll)]
                for st in sts:
                    _emit_front(tc, nc, st)
                for st in sts:
                    _emit_back(tc, nc, st)
                _release(sts)
    nc.compile()
    return nc


_NC_CACHE = None


def kernel(coefficients, knots, inpce, **run_kwargs):
    global _NC_CACHE
    if _NC_CACHE is None:
        _NC_CACHE = build_nc()
    nc = _NC_CACHE
    coefficients = np.ascontiguousarray(coefficients, dtype=np.float32)
    knots = np.ascontiguousarray(knots, dtype=np.float32)
    inpce = np.ascontiguousarray(inpce, dtype=np.float32)
    in_maps = []
    for k in range(NCORES):
        s = slice(k * BC, (k + 1) * BC)
        in_maps.append({"coefficients": coefficients[s],
                        "knots": knots[s],
                        "inpce": inpce[s]})
    res = run_bass_kernel_spmd(nc, in_maps, core_ids=list(range(NCORES)),
                               **run_kwargs)
    out = np.concatenate([res.results[k]["out"] for k in range(NCORES)], axis=0)
    if run_kwargs:
        return out, res
    return out


# revision 5
# speedup vs baseline: 1.6078x; 1.0671x over previous
"""Trainium2 Bass kernel for BSplineNN: cubic B-spline evaluation.

out[b, c] = sum_i coefficients[b, i, c] * N_{i,3}(x_b),  x_b = inpce[b, 0]

A cubic B-spline basis at one point has only 4 non-zero entries (rows
i0..i0+3, i0 = #{j in [4,64): t[j] <= x}), so per batch we fetch just the 4
relevant coefficient rows (4 KB) instead of all 64. Design notes, driven by
the TRN2 cost model (SWDGE desc-gen: 994 ns fixed + 0.34 ns/descriptor,
serial on the Pool engine; DMA ~22.5 B/ns; ~2 us HWDGE/sem latency per DMA):

* Only 4 SWDGE gathers (one per batch-group) with element-granular indices
  into a WIDE [BC, N*C] source view (axis=1 => index multiplier 1): the
  gathered 4 KB span always fits inside a source row, so each index emits
  ONE descriptor. (Indirect DMA consumes a single index per partition — a
  [P,G] index AP reads garbage on HW; and narrow [M,1] views fragment into
  one descriptor per element.)
* No knot-window gathers at all: kw[j] = t[i0+j] is selected ON-CHIP with a
  two-stage one-hot dot (coarse 16 blocks of 4, then fine 4), all exact
  integer-valued f32 compares. This removes 4 of the 8 desc-gens and their
  late DMA round trip.
* Group 0 is counted first so the serial desc-gen chain starts early; the
  basis runs under the coefficient DMA stream; contraction is spread over
  ACT (group-0 Copy-scale mults + Pool adds) and DVE (chains for groups
  1-3, last-arriving chunk last); three stores on alternating HWDGE rings.
* Pool's ALU has no comparison or per-partition-scalar ops (is_ge /
  TensorScalarPtr fail codegen) — it only gets plain add/sub/mult work.
* build_nc_loop software-pipelines the unrolled bodies (all loads, all
  fronts, all backs) so the in-order engine queues don't stall per body.

Sharding: pure data parallel, batch dim split across 8 cores (512 each).
Within a core, batch b = 4*p + g (p = partition 0..127, g = group 0..3).
"""

import numpy as np

import concourse.bacc as bacc
import concourse.bass as bass
import concourse.mybir as mybir
import concourse.tile as tile
from concourse.bass_utils import run_bass_kernel_spmd

B, N, C, T = 4096, 64, 256, 68   # batch, coef rows, channels, knots
K = 3                            # cubic
NCORES = 8
BC = B // NCORES                 # 512 batches per core
P = 128                          # partitions
G = BC // P                      # 4 batch-groups per partition
WROWS = K + 1                    # 4 gathered coef rows per batch
WK = 2 * K + 2                   # 8 window knots per batch
NMID = N - WROWS                 # 60 middle knots counted for i0
NQ = NMID + 1                    # 61 one-hot positions (i0 in [0, 60])
F32 = mybir.dt.float32
I32 = mybir.dt.int32
OP = mybir.AluOpType


def _emit_consts(nc, sb):
    """Loop-invariant index bases (f32 so the idx math stays float until the
    final int convert): bif = (4p+g)*N, and nothing else — kidx is gone."""
    bi = sb.tile([P, G], I32, tag="bi")
    nc.gpsimd.iota(out=bi[:], pattern=[[N, G]], base=0, channel_multiplier=N * G)
    bif256 = sb.tile([P, G], F32, tag="bif256")
    nc.vector.tensor_scalar(out=bif256[:], in0=bi[:], scalar1=float(C),
                            scalar2=None, op0=OP.mult)
    ones = sb.tile([P, G, WK - 1], F32, tag="ones")
    nc.vector.memset(ones[:], 1.0)
    agrid_i = sb.tile([P, G, 17], I32, tag="agrid_i")
    nc.gpsimd.iota(out=agrid_i[:], pattern=[[0, G], [4, 17]], base=0,
                   channel_multiplier=0)
    agrid = sb.tile([P, G, 17], F32, tag="agrid")
    nc.vector.tensor_copy(out=agrid[:], in_=agrid_i[:])
    bgrid_i = sb.tile([P, G, 5], I32, tag="bgrid_i")
    nc.gpsimd.iota(out=bgrid_i[:], pattern=[[0, G], [1, 5]], base=0,
                   channel_multiplier=0)
    bgrid = sb.tile([P, G, 5], F32, tag="bgrid")
    nc.vector.tensor_copy(out=bgrid[:], in_=bgrid_i[:])
    return bif256, ones, agrid, bgrid


def _emit_loads(tc, nc, coef, knots, inpce, out, bif256, ones, agrid, bgrid,
                tag=""):
    """Phase L: allocate this body's pool + tiles, issue the input DMAs."""
    pool_cm = tc.tile_pool(name=f"sb{tag}", bufs=1)
    sb = pool_cm.__enter__()
    st = {"pool_cm": pool_cm, "sb": sb,
          "args": (coef, knots, inpce, out, bif256, ones, agrid, bgrid)}
    # ---- load FULL knots (contiguous 1088 B per partition) + x ----
    ktf = sb.tile([P, G, T + 4], F32)
    nc.gpsimd.memset(ktf[:][:, :, T:T + 4], 0.0)
    kview = knots.rearrange("(p g) t -> p g t", g=G)
    nc.sync.dma_start(out=ktf[:][:, 0:1, 0:T], in_=kview[:, 0:1, :])
    nc.sync.dma_start(out=ktf[:][:, 1:G, 0:T], in_=kview[:, 1:G, :])
    xt = sb.tile([P, G], F32)
    nc.scalar.dma_start(out=xt[:], in_=inpce.rearrange("(p g) o -> p (g o)", g=G))
    st["ktf"], st["xt"] = ktf, xt
    return st


def _emit_front(tc, nc, st):
    """Phase F: count + gather indices + SWDGE gathers + window select +
    basis recurrence (everything up to the weights)."""
    sb = st["sb"]
    coef, knots, inpce, out, bif256, ones, agrid, bgrid = st["args"]
    ktf, xt = st["ktf"], st["xt"]
    if True:
        # ---- interval index i0 = #{j in [4,64): t[j] <= x} ----
        # group 0 counted FIRST so its gather index (and the serial Pool
        # desc-gen chain) starts ~0.6 us earlier; groups 1-3 follow while
        # desc-gen 0 runs.
        ind = sb.tile([P, G, NMID], F32)
        i0f = sb.tile([P, G], F32)
        gidx = sb.tile([P, G], I32)
        for gs in (slice(0, 1), slice(1, G)):
            w = gs.stop - gs.start
            nc.vector.tensor_tensor(out=ind[:][:, gs, :],
                                    in0=xt[:][:, gs].to_broadcast([P, w, NMID]),
                                    in1=ktf[:][:, gs, WROWS:N], op=OP.is_ge)
            nc.vector.reduce_sum(out=i0f[:][:, gs],
                                 in_=ind[:][:, gs, :],
                                 axis=mybir.AxisListType.X)
            # gidx = ((4p+g)*N + i0) * C via f32 (exact: max 8.4M < 2^24)
            nc.vector.scalar_tensor_tensor(out=gidx[:][:, gs], in0=i0f[:][:, gs],
                                           scalar=float(C), in1=bif256[:][:, gs],
                                           op0=OP.mult, op1=OP.add)

        # ---- the only SWDGE gathers: 4 coef chunks, wide-AP axis=1 ----
        gt = sb.tile([P, G, WROWS * C], F32)
        coef_wide = coef.rearrange("b n c -> b (n c)")
        order = []
        for g in range(G):
            order.append(nc.gpsimd.indirect_dma_start(
                out=gt[:][:, g, :], out_offset=None,
                in_=coef_wide,
                in_offset=bass.IndirectOffsetOnAxis(
                    ap=gidx[:][:, g:g + 1], axis=1)))
        for a, b in zip(order[1:], order):
            tile.add_dep_helper(a.ins, b.ins, sync=False,
                                reason="SWDGE emission order")

        # ---- knot windows on-chip, two-stage one-hot select (all DVE).
        # Stage 1: a0 = floor(i0/4); Da = one-hot over the 16 4-aligned
        # blocks; za[m] = sum_a ktf[4a+m]*Da[a] (m = 0..10) lifts an 11-knot
        # aligned window. Stage 2: b0 = i0 - 4*a0; Db = one-hot over 4;
        # kw[j] = sum_b za[b+j]*Db[b]. Indicator math is exact (integer-valued
        # f32 compares); the ktf pad is zeroed so Da's zero terms stay finite.
        inda = sb.tile([P, G, 17], F32)
        nc.vector.tensor_tensor(out=inda[:], in0=i0f[:].to_broadcast([P, G, 17]),
                                in1=agrid[:], op=OP.is_ge)
        Da = sb.tile([P, G, 16], F32)
        nc.vector.tensor_tensor(out=Da[:], in0=inda[:][:, :, 0:16],
                                in1=inda[:][:, :, 1:17], op=OP.subtract)
        a0f = sb.tile([P, G], F32)
        nc.vector.reduce_sum(out=a0f[:], in_=inda[:][:, :, 1:17],
                             axis=mybir.AxisListType.X)
        b0f = sb.tile([P, G], F32)
        nc.vector.scalar_tensor_tensor(out=b0f[:], in0=a0f[:], scalar=-4.0,
                                       in1=i0f[:], op0=OP.mult, op1=OP.add)
        indb = sb.tile([P, G, 5], F32)
        nc.vector.tensor_tensor(out=indb[:], in0=b0f[:].to_broadcast([P, G, 5]),
                                in1=bgrid[:], op=OP.is_ge)
        Db = sb.tile([P, G, 4], F32)
        nc.vector.tensor_tensor(out=Db[:], in0=indb[:][:, :, 0:4],
                                in1=indb[:][:, :, 1:5], op=OP.subtract)
        vv = ktf[:].rearrange("p g (a r) -> p g r a", r=4)  # [P, G, 4, 18]
        ZA = sb.tile([P, G, 11, 16], F32)
        for m in range(11):
            nc.vector.tensor_tensor(
                out=ZA[:][:, :, m, :],
                in0=vv[:, :, m % 4, m // 4:m // 4 + 16],
                in1=Da[:], op=OP.mult)
        za = sb.tile([P, G, 11], F32)
        nc.vector.reduce_sum(out=za[:], in_=ZA[:], axis=mybir.AxisListType.X)
        KW4 = sb.tile([P, G, WK, 4], F32)
        for b in range(4):
            nc.vector.tensor_tensor(
                out=KW4[:][:, :, :, b],
                in0=za[:][:, :, b:b + WK],
                in1=Db[:][:, :, b:b + 1].to_broadcast([P, G, WK]),
                op=OP.mult)
        kw = sb.tile([P, G, WK], F32)
        nc.vector.reduce_sum(out=kw[:], in_=KW4[:], axis=mybir.AxisListType.X)

        # ---- windowed Cox-de Boor, all 4 groups in one pass, prep work
        # (indicators, knot diffs, v, c-mults) on Pool, recip/u/a/add on DVE ----
        xb = xt[:].to_broadcast([P, G, WK])
        xmt = sb.tile([P, G, WK], F32)
        nc.vector.tensor_tensor(out=xmt[:], in0=xb, in1=kw[:], op=OP.subtract)
        ind = sb.tile([P, G, WK], F32)
        nc.vector.tensor_tensor(out=ind[:], in0=xb, in1=kw[:], op=OP.is_ge)
        lvl = sb.tile([P, G, WK - 1], F32, tag="lvl0")
        nc.vector.tensor_tensor(out=lvl[:], in0=ind[:][:, :, 0:WK - 1],
                                in1=ind[:][:, :, 1:WK], op=OP.subtract)
        us, vs = {}, {}
        for kk in (1, 2, 3):
            L1 = WK - kk          # number of u values = L + 1
            d = sb.tile([P, G, L1], F32, tag=f"d{kk}")
            nc.gpsimd.tensor_tensor(out=d[:], in0=kw[:][:, :, kk:kk + L1],
                                    in1=kw[:][:, :, 0:L1], op=OP.subtract)
            r = sb.tile([P, G, L1], F32, tag=f"r{kk}")
            nc.vector.reciprocal(out=r[:], in_=d[:])
            u = sb.tile([P, G, L1], F32, tag=f"u{kk}")
            nc.vector.tensor_tensor(out=u[:], in0=xmt[:][:, :, 0:L1],
                                    in1=r[:], op=OP.mult)
            v = sb.tile([P, G, L1], F32, tag=f"v{kk}")
            nc.gpsimd.tensor_tensor(out=v[:], in0=ones[:][:, :, 0:L1],
                                    in1=u[:], op=OP.subtract)
            us[kk], vs[kk] = u, v
        for kk in (1, 2, 3):
            L = WK - 1 - kk
            a = sb.tile([P, G, L], F32, tag=f"a{kk}")
            nc.vector.tensor_tensor(out=a[:], in0=us[kk][:][:, :, 0:L],
                                    in1=lvl[:][:, :, 0:L], op=OP.mult)
            c = sb.tile([P, G, L], F32, tag=f"c{kk}")
            nc.vector.tensor_tensor(out=c[:], in0=vs[kk][:][:, :, 1:L + 1],
                                    in1=lvl[:][:, :, 1:L + 1], op=OP.mult)
            nxt = sb.tile([P, G, L], F32, tag=f"lvl{kk}")
            nc.vector.tensor_tensor(out=nxt[:], in0=a[:], in1=c[:], op=OP.add)
            lvl = nxt
        wts = lvl  # [P, G, 4] basis weights for rows i0..i0+3
    st["gt"], st["wts"] = gt, wts
    return st


def _emit_back(tc, nc, st):
    """Phase B: contraction + stores; releases the body's pool."""
    sb = st["sb"]
    coef, knots, inpce, out, bif256, ones, agrid, bgrid = st["args"]
    gt, wts = st["gt"], st["wts"]
    if True:
        # ---- contraction: g0 = ACT Copy-scale mults + Pool adds (earliest
        # chunk, runs beside the DVE chains); g1-g3 = DVE weighted chains in
        # data-arrival order (g3 last). ----
        gtv = gt[:].rearrange("p g (d c) -> p g d c", d=WROWS)
        acc = sb.tile([P, G, C], F32)
        m0 = sb.tile([P, WROWS, C], F32)
        for d in range(WROWS):
            nc.scalar.activation(out=m0[:][:, d, :], in_=gtv[:, 0, d, :],
                                 func=mybir.ActivationFunctionType.Copy,
                                 scale=wts[:][:, 0, d:d + 1])
        s01 = sb.tile([P, C], F32)
        nc.gpsimd.tensor_tensor(out=s01[:], in0=m0[:][:, 0, :],
                                in1=m0[:][:, 1, :], op=OP.add)
        s23 = sb.tile([P, C], F32)
        nc.gpsimd.tensor_tensor(out=s23[:], in0=m0[:][:, 2, :],
                                in1=m0[:][:, 3, :], op=OP.add)
        nc.gpsimd.tensor_tensor(out=acc[:][:, 0, :], in0=s01[:], in1=s23[:],
                                op=OP.add)
        for g in (1, 2, 3):
            nc.vector.tensor_scalar_mul(out=acc[:][:, g, :], in0=gtv[:, g, 0, :],
                                        scalar1=wts[:][:, g, 0:1])
            for d in range(1, WROWS):
                nc.vector.scalar_tensor_tensor(
                    out=acc[:][:, g, :], in0=gtv[:, g, d, :],
                    scalar=wts[:][:, g, d:d + 1], in1=acc[:][:, g, :],
                    op0=OP.mult, op1=OP.add)

        # ---- stores: groups 0-1 together, then g2 and g3 as each chain
        # finishes (g3 rides the last coef chunk; a lone 1 KB store is the
        # shortest possible tail) ----
        outv = out.rearrange("(p g) c -> p g c", g=G)
        nc.sync.dma_start(out=outv[:, 1:3, :], in_=acc[:][:, 1:3, :])
        nc.scalar.dma_start(out=outv[:, 0:1, :], in_=acc[:][:, 0:1, :])
        nc.sync.dma_start(out=outv[:, 3:4, :], in_=acc[:][:, 3:4, :])


def _release(sts):
    """Pop the bodies' tile pools in reverse (stack) order."""
    for st in reversed(sts):
        st["pool_cm"].__exit__(None, None, None)


def _emit(tc, nc, coef, knots, inpce, out, bif256, ones, agrid, bgrid, tag=""):
    st = _emit_loads(tc, nc, coef, knots, inpce, out, bif256, ones, agrid,
                     bgrid, tag=tag)
    _emit_front(tc, nc, st)
    _emit_back(tc, nc, st)
    _release([st])


def build_nc(reps=1):
    nc = bacc.Bacc("TRN2", target_bir_lowering=False, debug=False,
                   num_devices=NCORES)
    coef = nc.dram_tensor("coefficients", [BC, N, C], F32, kind="ExternalInput")
    knots = nc.dram_tensor("knots", [BC, T], F32, kind="ExternalInput")
    inpce = nc.dram_tensor("inpce", [BC, 1], F32, kind="ExternalInput")
    out = nc.dram_tensor("out", [BC, C], F32, kind="ExternalOutput")
    with tile.TileContext(nc) as tc:
        with tc.tile_pool(name="const", bufs=1) as cpool:
            bif256, ones, agrid, bgrid = _emit_consts(nc, cpool)
            for r in range(reps):
                _emit(tc, nc, coef.ap(), knots.ap(), inpce.ap(), out.ap(),
                      bif256, ones, agrid, bgrid, tag=str(r))
    nc.compile()
    return nc


def build_nc_loop(trip, unroll=4):
    """Kernel body in a hardware For_i loop — benchmarking only. Body is
    unrolled `unroll` times on distinct tiles; divide the slope by unroll."""
    nc = bacc.Bacc("TRN2", target_bir_lowering=False, debug=False,
                   num_devices=NCORES)
    coef = nc.dram_tensor("coefficients", [BC, N, C], F32, kind="ExternalInput")
    knots = nc.dram_tensor("knots", [BC, T], F32, kind="ExternalInput")
    inpce = nc.dram_tensor("inpce", [BC, 1], F32, kind="ExternalInput")
    out = nc.dram_tensor("out", [BC, C], F32, kind="ExternalOutput")
    with tile.TileContext(nc) as tc:
        with tc.tile_pool(name="const", bufs=1) as cpool:
            bif256, ones, agrid, bgrid = _emit_consts(nc, cpool)
            with tc.For_i(0, trip, 1, staggered_reset=True):
                # software-pipeline the unrolled bodies: all loads, then all
                # fronts, then all backs, so the in-order engine queues never
                # stall on one body's DMA waits while another body has work.
                sts = [_emit_loads(tc, nc, coef.ap(), knots.ap(), inpce.ap(),
                                   out.ap(), bif256, ones, agrid, bgrid,
                                   tag=str(r))
                       for r in range(unroll)]
                for st in sts:
                    _emit_front(tc, nc, st)
                for st in sts:
                    _emit_back(tc, nc, st)
                _release(sts)
    nc.compile()
    return nc


_NC_CACHE = None


def kernel(coefficients, knots, inpce, **run_kwargs):
    global _NC_CACHE
    if _NC_CACHE is None:
        _NC_CACHE = build_nc()
    nc = _NC_CACHE
    coefficients = np.ascontiguousarray(coefficients, dtype=np.float32)
    knots = np.ascontiguousarray(knots, dtype=np.float32)
    inpce = np.ascontiguousarray(inpce, dtype=np.float32)
    in_maps = []
    for k in range(NCORES):
        s = slice(k * BC, (k + 1) * BC)
        in_maps.append({"coefficients": coefficients[s],
                        "knots": knots[s],
                        "inpce": inpce[s]})
    res = run_bass_kernel_spmd(nc, in_maps, core_ids=list(range(NCORES)),
                               **run_kwargs)
    out = np.concatenate([res.results[k]["out"] for k in range(NCORES)], axis=0)
    if run_kwargs:
        return out, res
    return out
